# revision 4
# baseline (speedup 1.0000x reference)
"""AdaptivePolyphaseSampling kernel for 8 TRN2 NeuronCores.

Reference semantics (STRIDE=2, P_NORM=2):
  x: [16, 96, 256, 256] f32
  poly[(i,j)] = x[:, :, i::2, j::2]           (4 components)
  norms[(i,j), b] = sum(poly^2 over C,H',W')  (monotone in p-norm)
  idx[b] = argmax over the 4 components
  out[b] = poly[idx[b], b]  -> [16, 96, 128, 128]

Sharding: pure data parallel over batch; 2 samples per core.

Per-core algorithm (per sample):
  Pass 1: stream x in channel-block chunks [128p=h', cblk*(2 rows)] and
    compute the 4 phase sums-of-squares with fused tensor_tensor_reduce
    (one DVE pass over the data).
  Argmax: reduce chunk partials, partition_all_reduce, compute the
    winner (i,j) as 0/1 flags entirely on-chip, load into registers.
  Pass 2: DMA only the winning h-parity rows (dynamic AP offset from
    the i register), column-subsample by j on the ACT engine (dynamic
    AP offset from the j register), DMA out.
"""

import numpy as np

import concourse.bass as bass
import concourse.bacc as bacc
import concourse.bass_isa as bass_isa
import concourse.mybir as mybir
import concourse.tile as tile
from concourse.bass import ds
from concourse.bass_utils import run_bass_kernel_spmd

N_CORES = 8
B = 16
C = 96
H = 256
W = 256
H2 = H // 2
W2 = W // 2
BPC = B // N_CORES  # samples per core

F32 = mybir.dt.float32
I32 = mybir.dt.int32

NCB = 8            # channels per pass-1 chunk
NCHUNK = C // NCB  # 12
NCB2 = 8           # channels per pass-2 chunk
NCHUNK2 = C // NCB2

# E[x^2]=1 for randn input; subtracting the expected per-partition sum
# before the cross-partition reduce keeps the accumulation near zero so
# fp32 rounding can't flip the argmax.
EXP_PART_SUM = float(C * H2 * W2 // 128 * 4)  # per-partition, all 4 phases... per phase:
EXP_PHASE_PART = float(C * H2 * W2 / 128)     # 12288.0 per partition per phase


import os
DBG = os.environ.get("APS_DEBUG", "full")  # full | static | nop2 | nop1


def build_kernel():
    nc = bacc.Bacc("TRN2", target_bir_lowering=False, debug=False,
                   num_devices=N_CORES)
    x_ext = nc.dram_tensor("x", [BPC, C, H, W], F32, kind="ExternalInput")
    out_ext = nc.dram_tensor("out", [BPC, C, H2, W2], F32, kind="ExternalOutput")

    with tile.TileContext(nc) as tc:
        _emit(tc, nc, x_ext, out_ext)
    nc.compile()
    return nc


def _emit(tc, nc, x_ext, out_ext):
    import contextlib
    ctx = contextlib.ExitStack()
    with ctx:
        p_in = ctx.enter_context(tc.tile_pool(name="p_in", bufs=3))
        p_sq = ctx.enter_context(tc.tile_pool(name="p_sq", bufs=1))
        p_acc = ctx.enter_context(tc.tile_pool(name="p_acc", bufs=1))
        p_small = ctx.enter_context(tc.tile_pool(name="p_small", bufs=1))
        p_row = ctx.enter_context(tc.tile_pool(name="p_row", bufs=3))
        p_out = ctx.enter_context(tc.tile_pool(name="p_out", bufs=3))

        # chunk partials: [128, sample, chunk, phase]
        acc = p_acc.tile([128, BPC * NCHUNK * 4], F32)
        sq = p_sq.tile([128, NCB * W2], F32)

        # constants for phase index extraction
        sel_i = p_small.tile([1, 4], F32, tag="sel_i")
        sel_j = p_small.tile([1, 4], F32, tag="sel_j")
        nc.vector.memset(sel_i[0:1, 0:2], 0.0)
        nc.vector.memset(sel_i[0:1, 2:4], 1.0)
        sjv = sel_j[0:1, 0:4].rearrange("p (w t) -> p w t", w=2, t=2)
        nc.vector.memset(sjv[:, :, 0:1], 0.0)
        nc.vector.memset(sjv[:, :, 1:2], 1.0)

        for s in range(BPC):
            # ---------- pass 1: phase sums of squares ----------
            for k in range(NCHUNK):
                c0 = k * NCB
                t = p_in.tile([128, NCB * 2 * W], F32, tag="in")
                src = x_ext[s, c0:c0 + NCB].rearrange(
                    "c (h2 i) w -> h2 c (i w)", i=2)
                tv = t[:].rearrange("p (c iw) -> p c iw", c=NCB)
                nc.sync.dma_start(tv, src)

                tp = t[:].rearrange("p (c i w2 j) -> p c i w2 j",
                                    c=NCB, i=2, w2=W2, j=2)
                sqv = sq[:].rearrange("p (c w2) -> p c w2", c=NCB)
                for ph in range(4):
                    i, j = ph // 2, ph % 2
                    col = (s * NCHUNK + k) * 4 + ph
                    nc.scalar.activation(
                        sqv, tp[:, :, i, :, j],
                        mybir.ActivationFunctionType.Square,
                        accum_out=acc[:, col:col + 1])

            if DBG == "nop1":
                continue
            # ---------- argmax ----------
            sums4 = p_small.tile([128, 4], F32, tag=f"sums4_{s}")
            accv = acc[:, s * NCHUNK * 4:(s + 1) * NCHUNK * 4].rearrange(
                "p (k f) -> p f k", k=NCHUNK, f=4)
            nc.vector.reduce_sum(sums4[:], accv, axis=mybir.AxisListType.X)
            # center before cross-partition accumulation (fp32 argmax safety)
            nc.vector.tensor_scalar(
                sums4[:], sums4[:], EXP_PHASE_PART, None,
                mybir.AluOpType.subtract)
            red4 = p_small.tile([128, 4], F32, tag=f"red4_{s}")
            nc.gpsimd.partition_all_reduce(
                red4[:], sums4[:], channels=128,
                reduce_op=bass_isa.ReduceOp.add)

            mx = p_small.tile([1, 1], F32, tag=f"mx_{s}")
            nc.vector.reduce_max(mx[0:1, :], red4[0:1, 0:4],
                                 axis=mybir.AxisListType.X)
            oh = p_small.tile([1, 4], F32, tag=f"oh_{s}")
            nc.vector.tensor_scalar(
                oh[0:1, :], red4[0:1, 0:4], mx[0:1, 0:1], None,
                mybir.AluOpType.is_equal)

            ij_f = p_small.tile([1, 2], F32, tag=f"ij_f_{s}")
            scr4 = p_small.tile([1, 4], F32, tag=f"scr4_{s}")
            nc.vector.tensor_tensor(
                out=scr4[0:1, :], in0=oh[0:1, :], in1=sel_i[0:1, :],
                op=mybir.AluOpType.mult)
            nc.vector.reduce_max(ij_f[0:1, 0:1], scr4[0:1, :],
                                 axis=mybir.AxisListType.X)
            nc.vector.tensor_tensor(
                out=scr4[0:1, :], in0=oh[0:1, :], in1=sel_j[0:1, :],
                op=mybir.AluOpType.mult)
            nc.vector.reduce_max(ij_f[0:1, 1:2], scr4[0:1, :],
                                 axis=mybir.AxisListType.X)

            ij_i = p_small.tile([1, 2], I32, tag=f"ij_i_{s}")
            nc.vector.tensor_copy(ij_i[0:1, :], ij_f[0:1, :])

            if DBG == "nop2":
                continue
            if DBG == "static":
                i_val, j_val = 0, 0
            else:
                i_val = nc.values_load(ij_i[0:1, 0:1], min_val=0, max_val=1,
                                       skip_runtime_bounds_check=True)
                j_val = nc.values_load(ij_i[0:1, 1:2], min_val=0, max_val=1,
                                       skip_runtime_bounds_check=True)

            # ---------- pass 2: gather the winning component ----------
            for k in range(NCHUNK2):
                c0 = k * NCB2
                r = p_row.tile([128, NCB2 * W], F32, tag="row")
                src = x_ext[s, c0:c0 + NCB2].rearrange(
                    "c (h2 i) w -> h2 c i w", i=2)[:, :, ds(i_val, 1), :]
                rv = r[:].rearrange("p (c i w) -> p c i w", c=NCB2, i=1)
                nc.sync.dma_start(rv, src)

                o = p_out.tile([128, NCB2 * W2], F32, tag="outt")
                rj = r[:].rearrange("p (c w2 j) -> p c w2 j",
                                    c=NCB2, w2=W2, j=2)[:, :, :, ds(j_val, 1)]
                ov = o[:].rearrange("p (c w2 j) -> p c w2 j",
                                    c=NCB2, w2=W2, j=1)
                nc.vector.tensor_copy(ov, rj)

                dst = out_ext[s, c0:c0 + NCB2].rearrange("c h2 w2 -> h2 c w2")
                nc.sync.dma_start(dst, o[:].rearrange("p (c w2) -> p c w2",
                                                      c=NCB2))


_NC = None


def _get_nc():
    global _NC
    if _NC is None:
        _NC = build_kernel()
    return _NC


def kernel(x: np.ndarray) -> np.ndarray:
    assert x.shape == (B, C, H, W) and x.dtype == np.float32
    nc = _get_nc()
    in_maps = [{"x": np.ascontiguousarray(x[c * BPC:(c + 1) * BPC])}
               for c in range(N_CORES)]
    res = run_bass_kernel_spmd(nc, in_maps, core_ids=list(range(N_CORES)))
    return np.concatenate([res.results[c]["out"] for c in range(N_CORES)],
                          axis=0)


# revision 5
# speedup vs baseline: 1.0786x; 1.0786x over previous
"""AdaptivePolyphaseSampling kernel for 8 TRN2 NeuronCores.

Reference semantics (STRIDE=2, P_NORM=2):
  x: [16, 96, 256, 256] f32
  poly[(i,j)] = x[:, :, i::2, j::2]           (4 components)
  norms[(i,j), b] = sum(poly^2 over C,H',W')  (monotone in p-norm)
  idx[b] = argmax over the 4 components
  out[b] = poly[idx[b], b]  -> [16, 96, 128, 128]

Sharding: pure data parallel over batch; 2 samples per core, no
communication.

Per-core algorithm (v1: full-sample SBUF residency):
  One sample (96ch x 512 f32/partition = 192KiB/partition) fits in SBUF
  (~208KiB usable). Stream the sample into 12 resident chunk tiles,
  square+accumulate phase sums on the ACT engine as chunks land, do the
  argmax fully on-chip, then select the winning polyphase component
  straight out of the resident tiles (dynamic AP offsets from
  registers) and DMA it out. x is read exactly once; no second pass.
  Sample 1 reuses the 12 chunk slots; its loads chase sample 0's
  selects chunk-by-chunk (Tile WAR deps), overlapping store of s0 with
  load of s1.
"""

import os

import numpy as np

import concourse.bass as bass
import concourse.bacc as bacc
import concourse.bass_isa as bass_isa
import concourse.mybir as mybir
import concourse.tile as tile
from concourse.bass import ds
from concourse.bass_utils import run_bass_kernel_spmd

N_CORES = 8
B = 16
C = 96
H = 256
W = 256
H2 = H // 2
W2 = W // 2
BPC = B // N_CORES  # samples per core

F32 = mybir.dt.float32
I32 = mybir.dt.int32

NCB = 8            # channels per resident chunk tile
NCHUNK = C // NCB  # 12 resident tiles
NCS = 4            # channels per ACT-square / select / out-DMA call
NSEG = C // NCS    # 24 accum segments per sample

# E[x^2]=1 for randn input; subtracting the expected per-sample-partition
# sum before the cross-partition reduce keeps the accumulation near zero
# so fp32 rounding cannot flip the argmax.
EXP_PHASE_PART = float(C * H2 * W2 / 128)  # 12288 per partition per phase


def build_kernel():
    nc = bacc.Bacc("TRN2", target_bir_lowering=False, debug=False,
                   num_devices=N_CORES)
    x_ext = nc.dram_tensor("x", [BPC, C, H, W], F32, kind="ExternalInput")
    out_ext = nc.dram_tensor("out", [BPC, C, H2, W2], F32, kind="ExternalOutput")

    with tile.TileContext(nc) as tc:
        _emit(tc, nc, x_ext, out_ext)
    nc.compile()
    return nc


def _emit(tc, nc, x_ext, out_ext):
    import contextlib
    ctx = contextlib.ExitStack()
    with ctx:
        p_res = ctx.enter_context(tc.tile_pool(name="p_res", bufs=NCHUNK))
        p_sq = ctx.enter_context(tc.tile_pool(name="p_sq", bufs=1))
        p_acc = ctx.enter_context(tc.tile_pool(name="p_acc", bufs=1))
        p_small = ctx.enter_context(tc.tile_pool(name="p_small", bufs=1))
        p_out = ctx.enter_context(tc.tile_pool(name="p_out", bufs=2))

        # per-(4ch-segment, phase) partials: [128, sample * seg * phase]
        acc = p_acc.tile([128, BPC * NSEG * 4], F32)
        sq = p_sq.tile([128, NCS * W2], F32)
        sqv = sq[:].rearrange("p (c w2) -> p c w2", c=NCS)

        # constants for phase index extraction
        sel_i = p_small.tile([1, 4], F32, tag="sel_i")
        sel_j = p_small.tile([1, 4], F32, tag="sel_j")
        nc.vector.memset(sel_i[0:1, 0:2], 0.0)
        nc.vector.memset(sel_i[0:1, 2:4], 1.0)
        sjv = sel_j[0:1, 0:4].rearrange("p (w t) -> p w t", w=2, t=2)
        nc.vector.memset(sjv[:, :, 0:1], 0.0)
        nc.vector.memset(sjv[:, :, 1:2], 1.0)

        for s in range(BPC):
            res = []
            # ---------- load resident + phase sums of squares ----------
            for k in range(NCHUNK):
                c0 = k * NCB
                t = p_res.tile([128, NCB * 2 * W], F32, tag="res")
                res.append(t)
                src = x_ext[s, c0:c0 + NCB].rearrange(
                    "c (h2 i) w -> h2 c (i w)", i=2)
                tv = t[:].rearrange("p (c iw) -> p c iw", c=NCB)
                nc.sync.dma_start(tv, src)

                tp = t[:].rearrange("p (c i w2 j) -> p c i w2 j",
                                    c=NCB, i=2, w2=W2, j=2)
                for half in range(NCB // NCS):
                    cs = half * NCS
                    seg = k * (NCB // NCS) + half
                    for ph in range(4):
                        i, j = ph // 2, ph % 2
                        col = (s * NSEG + seg) * 4 + ph
                        nc.scalar.activation(
                            sqv, tp[:, cs:cs + NCS, i, :, j],
                            mybir.ActivationFunctionType.Square,
                            accum_out=acc[:, col:col + 1])

            # ---------- argmax ----------
            sums4 = p_small.tile([128, 4], F32, tag=f"sums4_{s}")
            accv = acc[:, s * NSEG * 4:(s + 1) * NSEG * 4].rearrange(
                "p (k f) -> p f k", k=NSEG, f=4)
            nc.vector.reduce_sum(sums4[:], accv, axis=mybir.AxisListType.X)
            # center before cross-partition accumulation (fp32 argmax safety)
            nc.vector.tensor_scalar(
                sums4[:], sums4[:], EXP_PHASE_PART, None,
                mybir.AluOpType.subtract)
            red4 = p_small.tile([128, 4], F32, tag=f"red4_{s}")
            nc.gpsimd.partition_all_reduce(
                red4[:], sums4[:], channels=128,
                reduce_op=bass_isa.ReduceOp.add)

            mx = p_small.tile([1, 1], F32, tag=f"mx_{s}")
            nc.vector.reduce_max(mx[0:1, :], red4[0:1, 0:4],
                                 axis=mybir.AxisListType.X)
            oh = p_small.tile([1, 4], F32, tag=f"oh_{s}")
            nc.vector.tensor_scalar(
                oh[0:1, :], red4[0:1, 0:4], mx[0:1, 0:1], None,
                mybir.AluOpType.is_equal)

            ij_f = p_small.tile([1, 2], F32, tag=f"ij_f_{s}")
            scr4 = p_small.tile([1, 4], F32, tag=f"scr4_{s}")
            nc.vector.tensor_tensor(
                out=scr4[0:1, :], in0=oh[0:1, :], in1=sel_i[0:1, :],
                op=mybir.AluOpType.mult)
            nc.vector.reduce_max(ij_f[0:1, 0:1], scr4[0:1, :],
                                 axis=mybir.AxisListType.X)
            nc.vector.tensor_tensor(
                out=scr4[0:1, :], in0=oh[0:1, :], in1=sel_j[0:1, :],
                op=mybir.AluOpType.mult)
            nc.vector.reduce_max(ij_f[0:1, 1:2], scr4[0:1, :],
                                 axis=mybir.AxisListType.X)

            ij_i = p_small.tile([1, 2], I32, tag=f"ij_i_{s}")
            nc.vector.tensor_copy(ij_i[0:1, :], ij_f[0:1, :])

            i_val = nc.values_load(ij_i[0:1, 0:1], min_val=0, max_val=1,
                                   skip_runtime_bounds_check=True)
            j_val = nc.values_load(ij_i[0:1, 1:2], min_val=0, max_val=1,
                                   skip_runtime_bounds_check=True)

            # ---------- select winner straight from resident tiles ----------
            for k in range(NCHUNK):
                c0 = k * NCB
                tp = res[k][:].rearrange("p (c i w2 j) -> p c i w2 j",
                                         c=NCB, i=2, w2=W2, j=2)
                for half in range(NCB // NCS):
                    cs = half * NCS
                    o = p_out.tile([128, NCS * W2], F32, tag="outt")
                    src = tp[:, cs:cs + NCS, ds(i_val, 1), :, ds(j_val, 1)]
                    ov = o[:].rearrange("p (c i w2 j) -> p c i w2 j",
                                        c=NCS, i=1, w2=W2, j=1)
                    nc.vector.tensor_copy(ov, src)
                    dst = out_ext[s, c0 + cs:c0 + cs + NCS].rearrange(
                        "c h2 w2 -> h2 c w2")
                    nc.sync.dma_start(
                        dst, o[:].rearrange("p (c w2) -> p c w2", c=NCS))


_NC = None


def _get_nc():
    global _NC
    if _NC is None:
        _NC = build_kernel()
    return _NC


def kernel(x: np.ndarray) -> np.ndarray:
    assert x.shape == (B, C, H, W) and x.dtype == np.float32
    nc = _get_nc()
    in_maps = [{"x": np.ascontiguousarray(x[c * BPC:(c + 1) * BPC])}
               for c in range(N_CORES)]
    res = run_bass_kernel_spmd(nc, in_maps, core_ids=list(range(N_CORES)))
    return np.concatenate([res.results[c]["out"] for c in range(N_CORES)],
                          axis=0)


# revision 7
# speedup vs baseline: 1.2025x; 1.1149x over previous
"""AdaptivePolyphaseSampling kernel for 8 TRN2 NeuronCores.

Reference semantics (STRIDE=2, P_NORM=2):
  x: [16, 96, 256, 256] f32
  poly[(i,j)] = x[:, :, i::2, j::2]           (4 components)
  norms[(i,j), b] = sum(poly^2 over C,H',W')  (monotone in p-norm)
  idx[b] = argmax over the 4 components
  out[b] = poly[idx[b], b]  -> [16, 96, 128, 128]

Sharding: pure data parallel over batch; 2 samples per core, no
communication.

Per-core algorithm (v1: full-sample SBUF residency):
  One sample (96ch x 512 f32/partition = 192KiB/partition) fits in SBUF
  (~208KiB usable). Stream the sample into 12 resident chunk tiles,
  square+accumulate phase sums on the ACT engine as chunks land, do the
  argmax fully on-chip, then select the winning polyphase component
  straight out of the resident tiles (dynamic AP offsets from
  registers) and DMA it out. x is read exactly once; no second pass.
  Sample 1 reuses the 12 chunk slots; its loads chase sample 0's
  selects chunk-by-chunk (Tile WAR deps), overlapping store of s0 with
  load of s1.
"""

import os

import numpy as np

import concourse.bass as bass
import concourse.bacc as bacc
import concourse.bass_isa as bass_isa
import concourse.mybir as mybir
import concourse.tile as tile
from concourse.bass import ds
from concourse.bass_utils import run_bass_kernel_spmd

N_CORES = 8
B = 16
C = 96
H = 256
W = 256
H2 = H // 2
W2 = W // 2
BPC = B // N_CORES  # samples per core

F32 = mybir.dt.float32
I32 = mybir.dt.int32

NCB = 8            # channels per resident chunk tile
NCHUNK = C // NCB  # 12 resident tiles
NCS = 4            # channels per ACT-square / select / out-DMA call
NSEG = C // NCS    # 24 accum segments per sample

# E[x^2]=1 for randn input; subtracting the expected per-sample-partition
# sum before the cross-partition reduce keeps the accumulation near zero
# so fp32 rounding cannot flip the argmax.
EXP_PHASE_PART = float(C * H2 * W2 / 128)  # 12288 per partition per phase


def build_kernel():
    nc = bacc.Bacc("TRN2", target_bir_lowering=False, debug=False,
                   num_devices=N_CORES)
    x_ext = nc.dram_tensor("x", [BPC, C, H, W], F32, kind="ExternalInput")
    out_ext = nc.dram_tensor("out", [BPC, C, H2, W2], F32, kind="ExternalOutput")

    with tile.TileContext(nc) as tc:
        _emit(tc, nc, x_ext, out_ext)
    nc.compile()
    return nc


def _emit(tc, nc, x_ext, out_ext):
    import contextlib
    ctx = contextlib.ExitStack()
    with ctx:
        p_res = ctx.enter_context(tc.tile_pool(name="p_res", bufs=NCHUNK))
        p_psum = ctx.enter_context(
            tc.tile_pool(name="p_psum", bufs=2, space="PSUM"))
        p_acc = ctx.enter_context(tc.tile_pool(name="p_acc", bufs=1))
        p_small = ctx.enter_context(tc.tile_pool(name="p_small", bufs=1))
        p_out = ctx.enter_context(tc.tile_pool(name="p_out", bufs=2))

        # partials: per sample, 24 ACT cols (chunk x j, i=0 phases) then
        # 48 DVE cols (seg x j, i=1 phases)
        SACC = NCHUNK * 2 + NSEG * 2
        acc = p_acc.tile([128, BPC * SACC], F32)

        # constants for phase index extraction
        sel_i = p_small.tile([1, 4], F32, tag="sel_i")
        sel_j = p_small.tile([1, 4], F32, tag="sel_j")
        nc.vector.memset(sel_i[0:1, 0:2], 0.0)
        nc.vector.memset(sel_i[0:1, 2:4], 1.0)
        sjv = sel_j[0:1, 0:4].rearrange("p (w t) -> p w t", w=2, t=2)
        nc.vector.memset(sjv[:, :, 0:1], 0.0)
        nc.vector.memset(sjv[:, :, 1:2], 1.0)

        for s in range(BPC):
            res = []
            # ---------- load resident + phase sums of squares ----------
            for k in range(NCHUNK):
                c0 = k * NCB
                t = p_res.tile([128, NCB * 2 * W], F32, tag="res")
                res.append(t)
                src = x_ext[s, c0:c0 + NCB].rearrange(
                    "c (h2 i) w -> h2 c (i w)", i=2)
                tv = t[:].rearrange("p (c iw) -> p c iw", c=NCB)
                nc.sync.dma_start(tv, src)

                tp = t[:].rearrange("p (c i w2 j) -> p c i w2 j",
                                    c=NCB, i=2, w2=W2, j=2)
                for j in range(2):
                    # i=0 phases: ACT fused square+accumulate, whole chunk
                    col = s * SACC + k * 2 + j
                    sq_a = p_psum.tile([128, NCB * W2], F32, tag="sq_act")
                    nc.scalar.activation(
                        sq_a[:].rearrange("p (c w2) -> p c w2", c=NCB),
                        tp[:, :, 0, :, j],
                        mybir.ActivationFunctionType.Square,
                        accum_out=acc[:, col:col + 1])
                    # i=1 phases: DVE mult -> PSUM, then dense reduce
                    for half in range(NCB // NCS):
                        cs = half * NCS
                        col2 = (s * SACC + NCHUNK * 2
                                + (k * 2 + half) * 2 + j)
                        sq_d = p_psum.tile([128, NCS * W2], F32, tag="sq_dve")
                        nc.vector.tensor_tensor(
                            out=sq_d[:].rearrange("p (c w2) -> p c w2", c=NCS),
                            in0=tp[:, cs:cs + NCS, 1, :, j],
                            in1=tp[:, cs:cs + NCS, 1, :, j],
                            op=mybir.AluOpType.mult)
                        nc.vector.reduce_sum(
                            acc[:, col2:col2 + 1], sq_d[:],
                            axis=mybir.AxisListType.X)

            # ---------- argmax ----------
            sums4 = p_small.tile([128, 4], F32, tag=f"sums4_{s}")
            acc_a = acc[:, s * SACC:s * SACC + NCHUNK * 2].rearrange(
                "p (k j) -> p j k", k=NCHUNK, j=2)
            nc.vector.reduce_sum(sums4[:, 0:2], acc_a,
                                 axis=mybir.AxisListType.X)
            acc_d = acc[:, s * SACC + NCHUNK * 2:(s + 1) * SACC].rearrange(
                "p (k j) -> p j k", k=NSEG, j=2)
            nc.vector.reduce_sum(sums4[:, 2:4], acc_d,
                                 axis=mybir.AxisListType.X)
            # center before cross-partition accumulation (fp32 argmax safety)
            nc.vector.tensor_scalar(
                sums4[:], sums4[:], EXP_PHASE_PART, None,
                mybir.AluOpType.subtract)
            red4 = p_small.tile([128, 4], F32, tag=f"red4_{s}")
            nc.gpsimd.partition_all_reduce(
                red4[:], sums4[:], channels=128,
                reduce_op=bass_isa.ReduceOp.add)

            mx = p_small.tile([1, 1], F32, tag=f"mx_{s}")
            nc.vector.reduce_max(mx[0:1, :], red4[0:1, 0:4],
                                 axis=mybir.AxisListType.X)
            oh = p_small.tile([1, 4], F32, tag=f"oh_{s}")
            nc.vector.tensor_scalar(
                oh[0:1, :], red4[0:1, 0:4], mx[0:1, 0:1], None,
                mybir.AluOpType.is_equal)

            ij_f = p_small.tile([1, 2], F32, tag=f"ij_f_{s}")
            scr4 = p_small.tile([1, 4], F32, tag=f"scr4_{s}")
            nc.vector.tensor_tensor(
                out=scr4[0:1, :], in0=oh[0:1, :], in1=sel_i[0:1, :],
                op=mybir.AluOpType.mult)
            nc.vector.reduce_max(ij_f[0:1, 0:1], scr4[0:1, :],
                                 axis=mybir.AxisListType.X)
            nc.vector.tensor_tensor(
                out=scr4[0:1, :], in0=oh[0:1, :], in1=sel_j[0:1, :],
                op=mybir.AluOpType.mult)
            nc.vector.reduce_max(ij_f[0:1, 1:2], scr4[0:1, :],
                                 axis=mybir.AxisListType.X)

            ij_i = p_small.tile([1, 2], I32, tag=f"ij_i_{s}")
            nc.vector.tensor_copy(ij_i[0:1, :], ij_f[0:1, :])

            i_val = nc.values_load(ij_i[0:1, 0:1], min_val=0, max_val=1,
                                   skip_runtime_bounds_check=True)
            j_val = nc.values_load(ij_i[0:1, 1:2], min_val=0, max_val=1,
                                   skip_runtime_bounds_check=True)

            # ---------- select winner straight from resident tiles ----------
            for k in range(NCHUNK):
                c0 = k * NCB
                tp = res[k][:].rearrange("p (c i w2 j) -> p c i w2 j",
                                         c=NCB, i=2, w2=W2, j=2)
                for half in range(NCB // NCS):
                    cs = half * NCS
                    o = p_out.tile([128, NCS * W2], F32, tag="outt")
                    src = tp[:, cs:cs + NCS, ds(i_val, 1), :, ds(j_val, 1)]
                    ov = o[:].rearrange("p (c i w2 j) -> p c i w2 j",
                                        c=NCS, i=1, w2=W2, j=1)
                    nc.vector.tensor_copy(ov, src)
                    dst = out_ext[s, c0 + cs:c0 + cs + NCS].rearrange(
                        "c h2 w2 -> h2 c w2")
                    nc.sync.dma_start(
                        dst, o[:].rearrange("p (c w2) -> p c w2", c=NCS))


_NC = None


def _get_nc():
    global _NC
    if _NC is None:
        _NC = build_kernel()
    return _NC


def kernel(x: np.ndarray) -> np.ndarray:
    assert x.shape == (B, C, H, W) and x.dtype == np.float32
    nc = _get_nc()
    in_maps = [{"x": np.ascontiguousarray(x[c * BPC:(c + 1) * BPC])}
               for c in range(N_CORES)]
    res = run_bass_kernel_spmd(nc, in_maps, core_ids=list(range(N_CORES)))
    return np.concatenate([res.results[c]["out"] for c in range(N_CORES)],
                          axis=0)


# revision 9
# speedup vs baseline: 1.2797x; 1.0642x over previous
"""AdaptivePolyphaseSampling kernel for 8 TRN2 NeuronCores.

Reference semantics (STRIDE=2, P_NORM=2):
  x: [16, 96, 256, 256] f32
  poly[(i,j)] = x[:, :, i::2, j::2]           (4 components)
  norms[(i,j), b] = sum(poly^2 over C,H',W')  (monotone in p-norm)
  idx[b] = argmax over the 4 components
  out[b] = poly[idx[b], b]  -> [16, 96, 128, 128]

Sharding: pure data parallel over batch; 2 samples per core, no
communication.

Per-core algorithm (v1: full-sample SBUF residency):
  One sample (96ch x 512 f32/partition = 192KiB/partition) fits in SBUF
  (~208KiB usable). Stream the sample into 12 resident chunk tiles,
  square+accumulate phase sums on the ACT engine as chunks land, do the
  argmax fully on-chip, then select the winning polyphase component
  straight out of the resident tiles (dynamic AP offsets from
  registers) and DMA it out. x is read exactly once; no second pass.
  Sample 1 reuses the 12 chunk slots; its loads chase sample 0's
  selects chunk-by-chunk (Tile WAR deps), overlapping store of s0 with
  load of s1.
"""

import os

import numpy as np

import concourse.bass as bass
import concourse.bacc as bacc
import concourse.bass_isa as bass_isa
import concourse.mybir as mybir
import concourse.tile as tile
from concourse.bass import ds
from concourse.bass_utils import run_bass_kernel_spmd

N_CORES = 8
B = 16
C = 96
H = 256
W = 256
H2 = H // 2
W2 = W // 2
BPC = B // N_CORES  # samples per core

F32 = mybir.dt.float32
I32 = mybir.dt.int32

NCB = 8            # channels per resident chunk tile
NCHUNK = C // NCB  # 12 resident tiles
NCS = 4            # channels per ACT-square / select / out-DMA call
NSEG = C // NCS    # 24 accum segments per sample

# E[x^2]=1 for randn input; subtracting the expected per-sample-partition
# sum before the cross-partition reduce keeps the accumulation near zero
# so fp32 rounding cannot flip the argmax.
EXP_PHASE_PART = float(C * H2 * W2 / 128)  # 12288 per partition per phase


def build_kernel():
    nc = bacc.Bacc("TRN2", target_bir_lowering=False, debug=False,
                   num_devices=N_CORES)
    x_ext = nc.dram_tensor("x", [BPC, C, H, W], F32, kind="ExternalInput")
    out_ext = nc.dram_tensor("out", [BPC, C, H2, W2], F32, kind="ExternalOutput")

    with tile.TileContext(nc) as tc:
        _emit(tc, nc, x_ext, out_ext)
    nc.compile()
    return nc


def _emit(tc, nc, x_ext, out_ext):
    import contextlib
    ctx = contextlib.ExitStack()
    with ctx:
        p_res = ctx.enter_context(tc.tile_pool(name="p_res", bufs=NCHUNK))
        p_psum = ctx.enter_context(
            tc.tile_pool(name="p_psum", bufs=2, space="PSUM"))
        p_acc = ctx.enter_context(tc.tile_pool(name="p_acc", bufs=1))
        p_small = ctx.enter_context(tc.tile_pool(name="p_small", bufs=1))
        p_out = ctx.enter_context(tc.tile_pool(name="p_out", bufs=2))

        # partials: per sample, 36 ACT cols (chunk x ph{0,1,2}) then
        # 24 DVE cols (seg, ph3)
        SACC = NCHUNK * 3 + NSEG
        acc = p_acc.tile([128, BPC * SACC], F32)

        # constants for phase index extraction
        sel_i = p_small.tile([1, 4], F32, tag="sel_i")
        sel_j = p_small.tile([1, 4], F32, tag="sel_j")
        nc.vector.memset(sel_i[0:1, 0:2], 0.0)
        nc.vector.memset(sel_i[0:1, 2:4], 1.0)
        sjv = sel_j[0:1, 0:4].rearrange("p (w t) -> p w t", w=2, t=2)
        nc.vector.memset(sjv[:, :, 0:1], 0.0)
        nc.vector.memset(sjv[:, :, 1:2], 1.0)

        for s in range(BPC):
            res = []
            # ---------- load resident + phase sums of squares ----------
            for k in range(NCHUNK):
                c0 = k * NCB
                t = p_res.tile([128, NCB * 2 * W], F32, tag="res")
                res.append(t)
                src = x_ext[s, c0:c0 + NCB].rearrange(
                    "c (h2 i) w -> h2 c (i w)", i=2)
                tv = t[:].rearrange("p (c iw) -> p c iw", c=NCB)
                nc.sync.dma_start(tv, src)

                tp = t[:].rearrange("p (c i w2 j) -> p c i w2 j",
                                    c=NCB, i=2, w2=W2, j=2)
                # phases 0,1,2: ACT fused square+accumulate, whole chunk
                for t3, (pi, pj) in enumerate([(0, 0), (0, 1), (1, 0)]):
                    col = s * SACC + k * 3 + t3
                    sq_a = p_psum.tile([128, NCB * W2], F32, tag="sq_act")
                    nc.scalar.activation(
                        sq_a[:].rearrange("p (c w2) -> p c w2", c=NCB),
                        tp[:, :, pi, :, pj],
                        mybir.ActivationFunctionType.Square,
                        accum_out=acc[:, col:col + 1])
                # phase 3 (i=1,j=1): DVE mult -> PSUM, then dense reduce
                for half in range(NCB // NCS):
                    cs = half * NCS
                    col2 = s * SACC + NCHUNK * 3 + k * 2 + half
                    sq_d = p_psum.tile([128, NCS * W2], F32, tag="sq_dve")
                    nc.vector.tensor_tensor(
                        out=sq_d[:].rearrange("p (c w2) -> p c w2", c=NCS),
                        in0=tp[:, cs:cs + NCS, 1, :, 1],
                        in1=tp[:, cs:cs + NCS, 1, :, 1],
                        op=mybir.AluOpType.mult)
                    nc.vector.reduce_sum(
                        acc[:, col2:col2 + 1], sq_d[:],
                        axis=mybir.AxisListType.X)

            # ---------- argmax ----------
            sums4 = p_small.tile([128, 4], F32, tag=f"sums4_{s}")
            acc_a = acc[:, s * SACC:s * SACC + NCHUNK * 3].rearrange(
                "p (k t) -> p t k", k=NCHUNK, t=3)
            nc.vector.reduce_sum(sums4[:, 0:3], acc_a,
                                 axis=mybir.AxisListType.X)
            acc_d = acc[:, s * SACC + NCHUNK * 3:(s + 1) * SACC]
            nc.vector.reduce_sum(sums4[:, 3:4], acc_d,
                                 axis=mybir.AxisListType.X)
            # center before cross-partition accumulation (fp32 argmax safety)
            nc.vector.tensor_scalar(
                sums4[:], sums4[:], EXP_PHASE_PART, None,
                mybir.AluOpType.subtract)
            red4 = p_small.tile([128, 4], F32, tag=f"red4_{s}")
            nc.gpsimd.partition_all_reduce(
                red4[:], sums4[:], channels=128,
                reduce_op=bass_isa.ReduceOp.add)

            mx = p_small.tile([1, 1], F32, tag=f"mx_{s}")
            nc.vector.reduce_max(mx[0:1, :], red4[0:1, 0:4],
                                 axis=mybir.AxisListType.X)
            oh = p_small.tile([1, 4], F32, tag=f"oh_{s}")
            nc.vector.tensor_scalar(
                oh[0:1, :], red4[0:1, 0:4], mx[0:1, 0:1], None,
                mybir.AluOpType.is_equal)

            ij_f = p_small.tile([1, 2], F32, tag=f"ij_f_{s}")
            scr4 = p_small.tile([1, 4], F32, tag=f"scr4_{s}")
            nc.vector.tensor_tensor(
                out=scr4[0:1, :], in0=oh[0:1, :], in1=sel_i[0:1, :],
                op=mybir.AluOpType.mult)
            nc.vector.reduce_max(ij_f[0:1, 0:1], scr4[0:1, :],
                                 axis=mybir.AxisListType.X)
            nc.vector.tensor_tensor(
                out=scr4[0:1, :], in0=oh[0:1, :], in1=sel_j[0:1, :],
                op=mybir.AluOpType.mult)
            nc.vector.reduce_max(ij_f[0:1, 1:2], scr4[0:1, :],
                                 axis=mybir.AxisListType.X)

            ij_i = p_small.tile([1, 2], I32, tag=f"ij_i_{s}")
            nc.vector.tensor_copy(ij_i[0:1, :], ij_f[0:1, :])

            from concourse.ordered_set import OrderedSet
            veng = OrderedSet([mybir.EngineType.DVE])
            i_val = nc.values_load(ij_i[0:1, 0:1], engines=veng,
                                   min_val=0, max_val=1,
                                   skip_runtime_bounds_check=True)
            j_val = nc.values_load(ij_i[0:1, 1:2], engines=veng,
                                   min_val=0, max_val=1,
                                   skip_runtime_bounds_check=True)

            # ---------- select winner straight from resident tiles ----------
            for k in range(NCHUNK):
                c0 = k * NCB
                tp = res[k][:].rearrange("p (c i w2 j) -> p c i w2 j",
                                         c=NCB, i=2, w2=W2, j=2)
                for half in range(NCB // NCS):
                    cs = half * NCS
                    o = p_out.tile([128, NCS * W2], F32, tag="outt")
                    src = tp[:, cs:cs + NCS, ds(i_val, 1), :, ds(j_val, 1)]
                    ov = o[:].rearrange("p (c i w2 j) -> p c i w2 j",
                                        c=NCS, i=1, w2=W2, j=1)
                    nc.vector.tensor_copy(ov, src)
                    dst = out_ext[s, c0 + cs:c0 + cs + NCS].rearrange(
                        "c h2 w2 -> h2 c w2")
                    nc.sync.dma_start(
                        dst, o[:].rearrange("p (c w2) -> p c w2", c=NCS))


_NC = None


def _get_nc():
    global _NC
    if _NC is None:
        _NC = build_kernel()
    return _NC


def kernel(x: np.ndarray) -> np.ndarray:
    assert x.shape == (B, C, H, W) and x.dtype == np.float32
    nc = _get_nc()
    in_maps = [{"x": np.ascontiguousarray(x[c * BPC:(c + 1) * BPC])}
               for c in range(N_CORES)]
    res = run_bass_kernel_spmd(nc, in_maps, core_ids=list(range(N_CORES)))
    return np.concatenate([res.results[c]["out"] for c in range(N_CORES)],
                          axis=0)
